# revision 2
# baseline (speedup 1.0000x reference)
"""Trainium2 Bass kernel for per-edge-type Linear + ReLU (GNN message passing).

out[e] = relu(edge_features[e] @ W[edge_types[e]] + b[edge_types[e]])
E = 1M edges, D_in = D_out = 64, 8 edge types, 8 NeuronCores.

Strategy (sort-by-type on host; data-parallel over edges, weights replicated):
  - Host sorts edges by type (stable argsort) and deals each type's edges
    round-robin-ish across the 8 cores.  Every (core, type) pair gets a
    fixed-capacity segment of C edges (C = multiple of 512, chosen so
    C >= ceil(max_t count_t / 8)); short segments are zero-padded.  The
    padding is discarded on unshard.
  - Per-core device layout (all host-prepped, so the device does no
    gather/select work at all):
      * xt fp16 [128, 4*C]: partitions 0:64 hold X^T for the type-0..3
        segments side by side; partitions 64:128 hold X^T for types 4..7.
        128 partitions -> the load DMA uses all 16 SDMA engines.
      * wt fp16 [128, 256]: W[t] for t=0..3 at partitions 0:64 (64 cols
        each), W[t+4] at partitions 64:128.  lhsT = W directly since
        out_T = W^T @ x^T.
      * bt f32 [128, 4]: column s = [b[s] ; b[s+4]] stacked.
  - Per 512-edge group one matmul with the type's W held STATIONARY:
    quadrant (0,0) computes a type-s group into PSUM partitions 0:64,
    quadrant (64,64) computes a type-(s+4) group into partitions 64:128
    (tile_position is inferred from the operand base partitions).  One
    PSUM bank [128, 512] thus holds two groups = 1024 edges.
    No one-hot rows, no 8x overcompute, no max-reduce.
  - PSUM drain = fused bias + ReLU + fp16 cast, alternating between the
    vector engine (tensor_scalar add+max) and the scalar engine
    (activation Relu with per-partition bias) so neither becomes the
    bottleneck; stores stream out on the other HWDGE queue.
  - Output fp16 [nblk, 128, 2048]; host casts to f32, un-permutes and
    scatters rows back through the sort permutation.
"""

import os
from contextlib import ExitStack

import numpy as np

import concourse.bacc as bacc
import concourse.bass as bass
import concourse.mybir as mybir
import concourse.tile as tile
from concourse.bass_utils import run_bass_kernel_spmd

E_TOTAL = 1_000_000
D = 64
N_TYPES = 8
N_CORES = 8
GRP = 512               # edges per matmul / per PSUM half-tile
BLK_COLS = 2048         # SBUF macro-tile columns (per half: 4 groups -> 4096 edges)
PAIRS_PER_BLK = BLK_COLS // GRP  # 4 psum tiles per block

_BUILD_CACHE: dict = {}
LAST_RESULTS = None     # BassKernelResults from the most recent run (for test.py)


def _build_program(ec_pad: int, repeat: int = 1):
    """Build + compile the single-core Bass program (same on all 8 cores).

    ec_pad = 8 * C (total padded edges per core).  Requires C % 512 == 0.
    repeat > 1 wraps the block loop in a device-side For loop running the
    identical workload `repeat` times — used only for timing.
    """
    assert ec_pad % (2 * BLK_COLS) == 0
    cap = ec_pad // N_TYPES          # C: edges per (core, type) segment
    assert cap % GRP == 0
    q = cap // GRP                   # groups per segment
    half_cols = 4 * cap              # columns per partition-half
    nblk = half_cols // BLK_COLS
    f16 = mybir.dt.float16
    f32 = mybir.dt.float32

    nc = bacc.Bacc("TRN2", target_bir_lowering=False, debug=False)

    xt = nc.dram_tensor("xt", [2 * D, half_cols], f16, kind="ExternalInput").ap()
    wt = nc.dram_tensor("wt", [2 * D, 4 * D], f16, kind="ExternalInput").ap()
    bt = nc.dram_tensor("bt", [2 * D, 4], f32, kind="ExternalInput").ap()
    out = nc.dram_tensor("out", [nblk, 2 * D, BLK_COLS], f16, kind="ExternalOutput").ap()

    with tile.TileContext(nc) as tc, ExitStack() as ctx:
        const_pool = ctx.enter_context(tc.tile_pool(name="consts", bufs=1))
        xt_pool = ctx.enter_context(tc.tile_pool(name="xt", bufs=4))
        out_pool = ctx.enter_context(tc.tile_pool(name="outs", bufs=4))
        z_pool = ctx.enter_context(tc.tile_pool(name="z", bufs=6, space="PSUM"))

        wt_sb = const_pool.tile([2 * D, 4 * D], f16)
        bt_sb = const_pool.tile([2 * D, 4], f32)
        nc.sync.dma_start(wt_sb[:], wt)
        nc.sync.dma_start(bt_sb[:], bt)

        rep_ctx = tc.For_i(0, repeat, 1) if repeat > 1 else None
        if rep_ctx is not None:
            rep_ctx.__enter__()

        for blk in range(nblk):
            sl = slice(blk * BLK_COLS, (blk + 1) * BLK_COLS)
            xt_t = xt_pool.tile([2 * D, BLK_COLS], f16, tag="xt")
            nc.sync.dma_start(xt_t[:], xt[:, sl])

            out_t = out_pool.tile([2 * D, BLK_COLS], f16, tag="outs")
            for jj in range(PAIRS_PER_BLK):
                g = blk * PAIRS_PER_BLK + jj   # group index within the half
                s = g // q                     # segment 0..3 (type s on top, s+4 below)
                js = slice(jj * GRP, (jj + 1) * GRP)
                z = z_pool.tile([2 * D, GRP], f32, tag="z")
                # Two PE quadrants, two independent 512-edge groups.
                nc.tensor.matmul(
                    z[0:D, :], lhsT=wt_sb[0:D, s * D : (s + 1) * D],
                    rhs=xt_t[0:D, js], start=True, stop=True,
                )
                nc.tensor.matmul(
                    z[D : 2 * D, :], lhsT=wt_sb[D : 2 * D, s * D : (s + 1) * D],
                    rhs=xt_t[D : 2 * D, js], start=True, stop=True,
                )
                # Fused bias + ReLU + fp16 cast, alternating DVE / ACT.
                if jj % 2 == 0:
                    nc.vector.tensor_scalar(
                        out=out_t[:, js], in0=z[:],
                        scalar1=bt_sb[:, s : s + 1], scalar2=0.0,
                        op0=mybir.AluOpType.add, op1=mybir.AluOpType.max,
                    )
                else:
                    nc.scalar.activation(
                        out_t[:, js], z[:],
                        mybir.ActivationFunctionType.Relu,
                        bias=bt_sb[:, s : s + 1], scale=1.0,
                    )

            nc.scalar.dma_start(out[blk], out_t[:])

        if rep_ctx is not None:
            rep_ctx.__exit__(None, None, None)

    nc.compile()
    return nc


def _get_program(ec_pad: int):
    if ec_pad not in _BUILD_CACHE:
        _BUILD_CACHE[ec_pad] = _build_program(ec_pad)
    return _BUILD_CACHE[ec_pad]


def _plan(edge_types):
    """Host-side shard plan: per (core, type) lists of edge indices + capacity."""
    t_idx = np.asarray(edge_types).astype(np.int64)
    e_total = t_idx.shape[0]
    order = np.argsort(t_idx, kind="stable")
    counts = np.bincount(t_idx, minlength=N_TYPES)
    # capacity: max per-core share, rounded up to a 512-multiple (and at
    # least 2 blocks' worth so the block loop is non-degenerate)
    max_share = int(np.ceil(counts.max() / N_CORES))
    cap = max(((max_share + GRP - 1) // GRP) * GRP, BLK_COLS)
    chunks = {}  # (core, type) -> index array
    off = 0
    for t in range(N_TYPES):
        idx_t = order[off : off + counts[t]]
        off += counts[t]
        qd, r = divmod(len(idx_t), N_CORES)
        pos = 0
        for c in range(N_CORES):
            n = qd + (1 if c < r else 0)
            chunks[(c, t)] = idx_t[pos : pos + n]
            pos += n
    return chunks, cap, e_total


def build_in_maps(edge_features, edge_types, W, b):
    chunks, cap, e_total = _plan(edge_types)
    x16 = np.asarray(edge_features, dtype=np.float16)

    wt = np.zeros((2 * D, 4 * D), dtype=np.float16)
    bt = np.zeros((2 * D, 4), dtype=np.float32)
    Wf = np.asarray(W, dtype=np.float16)
    bf = np.asarray(b, dtype=np.float32)
    for s in range(4):
        wt[0:D, s * D : (s + 1) * D] = Wf[s]
        wt[D : 2 * D, s * D : (s + 1) * D] = Wf[s + 4]
        bt[0:D, s] = bf[s]
        bt[D : 2 * D, s] = bf[s + 4]

    half_cols = 4 * cap
    in_maps = []
    for c in range(N_CORES):
        xt = np.zeros((2 * D, half_cols), dtype=np.float16)
        for t in range(N_TYPES):
            idx = chunks[(c, t)]
            row0 = 0 if t < 4 else D
            col0 = (t % 4) * cap
            xt[row0 : row0 + D, col0 : col0 + len(idx)] = x16[idx].T
        in_maps.append({"xt": xt, "wt": wt, "bt": bt})
    return in_maps


def _unpack_out(arr, chunks, c, cap):
    """[nblk, 128, 2048] fp16 -> (segment-ordered rows [8*cap, 64] f32)."""
    nblk = arr.shape[0]
    # [nblk, half(2), dout(64), pair(4), col(512)] -> [half, nblk, pair, col, dout]
    a = arr.reshape(nblk, 2, D, PAIRS_PER_BLK, GRP).transpose(1, 0, 3, 4, 2)
    return a.reshape(2, nblk * BLK_COLS, D)  # [half, 4*cap, 64]


def kernel(edge_features, edge_types, W, b):
    global LAST_RESULTS
    e_total = edge_features.shape[0]
    chunks, cap, _ = _plan(edge_types)
    ec_pad = N_TYPES * cap

    nc = _get_program(ec_pad)
    in_maps = build_in_maps(edge_features, edge_types, W, b)

    res = run_bass_kernel_spmd(
        nc,
        in_maps,
        core_ids=list(range(N_CORES)),
        trace=bool(int(os.environ.get("EDGE_KERNEL_TRACE", "0"))),
    )
    LAST_RESULTS = res

    out = np.empty((e_total, D), dtype=np.float32)
    for c in range(N_CORES):
        halves = _unpack_out(res.results[c]["out"], chunks, c, cap)
        for t in range(N_TYPES):
            idx = chunks[(c, t)]
            col0 = (t % 4) * cap
            seg = halves[t // 4, col0 : col0 + len(idx), :]
            out[idx] = seg.astype(np.float32)
    return out
